# revision 2
# baseline (speedup 1.0000x reference)
import numpy as np
import concourse.bass as bass
import concourse.tile as tile
from concourse import bacc, mybir
from concourse.bass_utils import run_bass_kernel_spmd

# Block self-attention: 32x32 areas of 4x4 blocks of 8x8 pixels.
# Sharding: 8 cores = 4 batches x 2 H-halves of 256 rows (8 area-rows).
# Padding trick: host pads x spatially with the vector xpad solving
# w_ptg @ xpad + b_ptg = 0, so conv1 output is exactly 0 at padded pixels
# (matches reference, which zero-pads after conv+bias).

F32 = mybir.dt.float32
F32R = mybir.dt.float32r
MM_DT = F32

_cached = {}


def _build_nc():
    nc = bacc.Bacc("TRN2", target_bir_lowering=False, debug=False, num_devices=8)
    xs = nc.dram_tensor("xs", [64, 256, 512], F32, kind="ExternalInput").ap()
    w1t = nc.dram_tensor("w1t", [64, 48], F32, kind="ExternalInput").ap()
    b1 = nc.dram_tensor("b1", [48, 1], F32, kind="ExternalInput").ap()
    w2t = nc.dram_tensor("w2t", [16, 64], F32, kind="ExternalInput").ap()
    b2 = nc.dram_tensor("b2", [64, 1], F32, kind="ExternalInput").ap()
    ident = nc.dram_tensor("ident", [128, 128], F32, kind="ExternalInput").ap()
    mask = nc.dram_tensor("mask", [128, 128], F32, kind="ExternalInput").ap()
    out = nc.dram_tensor("out", [64, 256, 512], F32, kind="ExternalOutput").ap()

    def mm(o, l, r, **kw):
        if MM_DT is not F32:
            l = l.bitcast(MM_DT)
            r = r.bitcast(MM_DT)
        nc.tensor.matmul(o, l, r, **kw)

    with tile.TileContext(nc) as tc:
        with (
            tc.tile_pool(name="const", bufs=1) as cpool,
            tc.tile_pool(name="xy", bufs=1) as xy,
            tc.tile_pool(name="mid", bufs=2) as mid,
            tc.tile_pool(name="sm", bufs=3) as sm,
            tc.tile_pool(name="ps", bufs=3, space="PSUM") as ps,
        ):
            w1_t = cpool.tile([64, 48], F32)
            nc.sync.dma_start(w1_t, w1t)
            b1_t = cpool.tile([48, 1], F32)
            nc.sync.dma_start(b1_t, b1)
            w2_t = cpool.tile([16, 64], F32)
            nc.sync.dma_start(w2_t, w2t)
            b2_t = cpool.tile([64, 1], F32)
            nc.sync.dma_start(b2_t, b2)
            id_t = cpool.tile([128, 128], F32)
            nc.sync.dma_start(id_t, ident)
            mk_t = cpool.tile([128, 128], F32)
            nc.sync.dma_start(mk_t, mask)

            for s in range(8):
                for gw in range(2):
                    # half strip: 32 rows x 256 cols = 8 areas
                    x_t = xy.tile([64, 32 * 256], F32, tag="x")
                    nc.sync.dma_start(
                        x_t,
                        xs[:, 32 * s : 32 * s + 32, 256 * gw : 256 * gw + 256],
                    )
                    # block-ordered view: (c, ih, aw, iw, ph, pw)
                    xv = x_t.rearrange(
                        "c (ih ph aw iw pw) -> c ih aw iw ph pw",
                        ih=4, ph=8, aw=8, iw=4, pw=8,
                    )
                    y_t = xy.tile([48, 8192], F32, tag="y")
                    for a in range(8):
                        for ih in range(4):
                            p1 = ps.tile([48, 256], F32, tag="mm")
                            mm(p1, w1_t, xv[:, ih, a], start=True, stop=True)
                            off = a * 1024 + ih * 256
                            nc.scalar.activation(
                                y_t[:, off : off + 256],
                                p1, mybir.ActivationFunctionType.Identity, bias=b1_t,
                            )
                    # y free index = aw*1024 + i*64 + p (block-linear)
                    qkc = mid.tile([128, 2048], F32, tag="qkc")
                    qv = qkc.rearrange("ai (c p) -> ai c p", c=32, p=64)
                    for c in range(32):
                        nc.sync.dma_start(qv[:, c], y_t[c : c + 1, :])
                    gc = mid.tile([128, 1024], F32, tag="gc")
                    gv = gc.rearrange("ai (c p) -> ai c p", c=16, p=64)
                    for c in range(16):
                        nc.sync.dma_start(gv[:, c], y_t[32 + c : 33 + c, :])
                    qkb = mid.tile([128, 2048], F32, tag="qkb")
                    for q in range(16):
                        tp = ps.tile([128, 128], F32, tag="tp")
                        nc.tensor.transpose(tp, qkc[:, 128 * q : 128 * q + 128], id_t)
                        sl = qkb[:, 128 * q : 128 * q + 128]
                        if q % 2 == 0:
                            nc.scalar.activation(
                                sl, tp, mybir.ActivationFunctionType.Copy
                            )
                        else:
                            nc.vector.tensor_copy(sl, tp)
                    sps = ps.tile([128, 128], F32, tag="mm")
                    for k in range(8):
                        mm(
                            sps,
                            qkb[:, 128 * k : 128 * k + 128],
                            qkb[:, 1024 + 128 * k : 1024 + 128 * k + 128],
                            start=(k == 0), stop=(k == 7),
                        )
                    e_t = sm.tile([128, 128], F32, tag="e")
                    nc.scalar.activation(e_t, sps, mybir.ActivationFunctionType.Exp)
                    nc.vector.tensor_mul(e_t, e_t, mk_t)
                    r_t = sm.tile([128, 1], F32, tag="r")
                    nc.vector.reduce_sum(r_t, e_t, axis=mybir.AxisListType.X)
                    nc.vector.reciprocal(r_t, r_t)
                    p_t = sm.tile([128, 128], F32, tag="p")
                    nc.vector.tensor_scalar_mul(p_t, e_t, r_t)
                    ptp = ps.tile([128, 128], F32, tag="tp")
                    nc.tensor.transpose(ptp, p_t, id_t)
                    pT = sm.tile([128, 128], F32, tag="pT")
                    nc.scalar.activation(pT, ptp, mybir.ActivationFunctionType.Copy)
                    o_c = mid.tile([128, 1024], F32, tag="oc")
                    for h in range(2):
                        op = ps.tile([128, 512], F32, tag="mm")
                        mm(op, pT, gc[:, 512 * h : 512 * h + 512],
                           start=True, stop=True)
                        sl = o_c[:, 512 * h : 512 * h + 512]
                        if h == 0:
                            nc.scalar.activation(
                                sl, op, mybir.ActivationFunctionType.Copy
                            )
                        else:
                            nc.vector.tensor_copy(sl, op)
                    ost = xy.tile([16, 8192], F32, tag="ost")
                    ocv = o_c.rearrange("ai (c p) -> ai c p", c=16, p=64)
                    for c in range(16):
                        nc.sync.dma_start(ost[c : c + 1], ocv[:, c])
                    osum = xy.tile([64, 8192], F32, tag="osum")
                    # image-ordered view: flat = (ih*8+ph)*256 + a*32 + iw*8 + pw
                    osv = osum.rearrange(
                        "c (ih ph aw iw pw) -> c ih aw ph iw pw",
                        ih=4, ph=8, aw=8, iw=4, pw=8,
                    )
                    for a in range(8):
                        for ih in range(4):
                            off = a * 1024 + ih * 256
                            p2 = ps.tile([64, 256], F32, tag="mm")
                            mm(p2, w2_t, ost[:, off : off + 256],
                               start=True, stop=False)
                            mm(p2, id_t[0:64, 0:64], xv[:, ih, a],
                               start=False, stop=True)
                            p2v = p2.rearrange(
                                "c (iw ph pw) -> c ph iw pw", iw=4, ph=8, pw=8
                            )
                            nc.scalar.activation(
                                osv[:, ih, a],
                                p2v, mybir.ActivationFunctionType.Identity, bias=b2_t,
                            )
                    nc.sync.dma_start(
                        out[:, 32 * s : 32 * s + 32, 256 * gw : 256 * gw + 256],
                        osum,
                    )
    nc.compile()
    return nc


def kernel(x, w_ptg, b_ptg, w_out, b_out):
    x = np.asarray(x, dtype=np.float32)
    w_ptg = np.asarray(w_ptg, dtype=np.float32)
    b_ptg = np.asarray(b_ptg, dtype=np.float32)
    w_out = np.asarray(w_out, dtype=np.float32)
    b_out = np.asarray(b_out, dtype=np.float32)

    # pad vector: w_ptg @ xpad + b_ptg = 0
    xpad, *_ = np.linalg.lstsq(w_ptg, -b_ptg, rcond=None)
    xp = np.empty((4, 64, 512, 512), np.float32)
    xp[:] = xpad.astype(np.float32)[None, :, None, None]
    xp[:, :, :504, :504] = x

    ident = np.eye(128, dtype=np.float32)
    mask = np.zeros((128, 128), np.float32)
    for a in range(8):
        mask[16 * a : 16 * a + 16, 16 * a : 16 * a + 16] = 1.0

    common = {
        "w1t": np.ascontiguousarray(w_ptg.T),
        "b1": np.ascontiguousarray(b_ptg[:, None]),
        "w2t": np.ascontiguousarray(w_out.T),
        "b2": np.ascontiguousarray(b_out[:, None]),
        "ident": ident,
        "mask": mask,
    }
    in_maps = []
    for b in range(4):
        for h in range(2):
            in_maps.append(
                {"xs": np.ascontiguousarray(xp[b, :, 256 * h : 256 * h + 256, :]),
                 **common}
            )

    if "nc" not in _cached:
        _cached["nc"] = _build_nc()
    res = run_bass_kernel_spmd(_cached["nc"], in_maps, list(range(8)))
    _cached["res"] = res

    outp = np.empty((4, 64, 512, 512), np.float32)
    for i in range(8):
        b, h = divmod(i, 2)
        outp[b, :, 256 * h : 256 * h + 256, :] = res.results[i]["out"]
    return np.ascontiguousarray(outp[:, :, :504, :504])


if __name__ == "__main__":
    import reference

    inputs = {k: np.asarray(v) for k, v in reference.setup_inputs().items()}
    got = kernel(**inputs)
    exp = np.asarray(reference.reference(**inputs))
    err = np.abs(got - exp).max() / np.abs(exp).max()
    print("Relative error:", err)

